# revision 2
# baseline (speedup 1.0000x reference)
"""Trainium2 Bass kernel for the ExpCloudMMD loss.

reference math (gamma = 0.5):
  t1 = mean_{j,k} exp(-g*||p_j - p_k||^2)            over [8192, 8192]
  t2 = 2/(Nx*Np) * sum_{i,j} exp(-g*||x_i - p_j||^2) over [32768, 8192]
  out = t1 - t2  (f32 scalar)

Strategy (8 cores, SPMD, no collectives):
  - t2: shard x rows 8-way; each core computes its 4096x8192 cross block.
  - t1: the particle Gram is symmetric; in 2048x2048 super-blocks only the
    diagonal (4) + strict upper (6) of the 4x4 grid are computed, and the
    host doubles the upper sums. The 160 (row-block, col-group) pairs are
    dealt round-robin to the 8 cores; each core's pair list rides in its
    `pslhs` input tensor, so the program stays identical across cores.
  - The exp *argument* p.x - g|x|^2 - g|p|^2 is produced directly by a
    single K=68 matmul per output tile using an augmented bf16 hi/lo
    encoding (4-way split product + norm channels), so ScalarE needs no
    bias and the pipeline per [128, 2048] PSUM group is:
        4x matmul (PE) -> 1x activation(Exp, accum_out) (ACT)
    ACT is the roofline engine (~1 elem/lane/cycle @ 1.2 GHz).
  - Each ACT writes its partial row-sums into one column of a [128, 148]
    SBUF accumulator; the accumulator is DMA'd out and the final (tiny)
    weighted reduction + scaling happens on the host in float64.
"""

import threading

import ml_dtypes
import numpy as np

import concourse.bass as bass  # noqa: F401
import concourse.mybir as mybir
import concourse.tile as tile
from concourse import bacc, bass_utils

bf16 = ml_dtypes.bfloat16

GAMMA = 0.5
NX, NP, D = 32768, 8192, 16
N_CORES = 8
XS = NX // N_CORES     # 4096 x rows per core
K = 68                 # 4*16 (hi/lo product blocks) + 2 + 2 norm channels

# t1 coarse-triangle schedule: for col-super-group g (2048 particles),
# the computed row-blocks are the 16*(g+1) blocks of super-rows 0..g,
# dealt round-robin (r % 8) to cores -> per-core counts 2,4,6,8.
T1_COUNTS = [2, 4, 6, 8]
N_T1_GROUPS = sum(T1_COUNTS)                   # 20 per core
N_CROSS_GROUPS = (NP // 128) * (XS // 2048)    # 64 * 2 = 128 per core
N_GROUPS = N_CROSS_GROUPS + N_T1_GROUPS        # 148
PS_COLS = N_T1_GROUPS * 128                    # 2560 pslhs columns per core

N_PCHUNK = 8  # plhs load chunks (8 j-blocks each) for early compute start


def _t1_pairs(core):
    """[(row_block, col_group, weight)] for this core, in program order."""
    pairs = []
    for g in range(4):
        rows = [r for r in range(16 * (g + 1)) if r % N_CORES == core]
        assert len(rows) == T1_COUNTS[g]
        for r in rows:
            pairs.append((r, g, 1.0 if r // 16 == g else 2.0))
    return pairs


def _build_nc():
    nc = bacc.Bacc(
        "TRN2",
        target_bir_lowering=False,
        debug=False,
        enable_asserts=False,
        num_devices=N_CORES,
    )
    dt = mybir.dt
    plhs = nc.dram_tensor("plhs", [K, NP], dt.bfloat16, kind="ExternalInput").ap()
    prhs = nc.dram_tensor("prhs", [K, NP], dt.bfloat16, kind="ExternalInput").ap()
    xrhs = nc.dram_tensor("xrhs", [K, XS], dt.bfloat16, kind="ExternalInput").ap()
    pslhs = nc.dram_tensor("pslhs", [K, PS_COLS], dt.bfloat16, kind="ExternalInput").ap()
    acc_d = nc.dram_tensor("acc", [128, N_GROUPS], dt.float32, kind="ExternalOutput").ap()

    with tile.TileContext(nc) as tc:
        with (
            tc.tile_pool(name="const", bufs=1) as const,
            tc.tile_pool(name="scrp", bufs=2) as scrp,
            tc.tile_pool(name="psp", bufs=2, space="PSUM") as psp,
        ):
            sb_plhs = const.tile([K, NP], dt.bfloat16)
            sb_prhs = const.tile([K, NP], dt.bfloat16)
            sb_xrhs = const.tile([K, XS], dt.bfloat16)
            sb_pslhs = const.tile([K, PS_COLS], dt.bfloat16)
            sb_acc = const.tile([128, N_GROUPS], dt.float32)
            sb_tiny = const.tile([1, 1], dt.float32)

            # Warm the ACT exp table set (~2.7us) during the DMA prologue.
            nc.gpsimd.memset(sb_tiny[:], 0.0)
            nc.scalar.activation(
                sb_tiny[:], sb_tiny[:], mybir.ActivationFunctionType.Exp
            )

            # Input loads, in consumption order. The first matmul only
            # needs plhs chunk 0 + the first xrhs half.
            pchunk = NP // N_PCHUNK
            nc.sync.dma_start(sb_plhs[:, 0:pchunk], plhs[:, 0:pchunk])
            nc.sync.dma_start(sb_xrhs[:, 0:2048], xrhs[:, 0:2048])
            nc.sync.dma_start(sb_xrhs[:, 2048:XS], xrhs[:, 2048:XS])
            for i in range(1, N_PCHUNK):
                s = slice(i * pchunk, (i + 1) * pchunk)
                nc.sync.dma_start(sb_plhs[:, s], plhs[:, s])
            nc.sync.dma_start(sb_pslhs[:], pslhs[:])
            nc.sync.dma_start(sb_prhs[:], prhs[:])

            col = 0

            def group(lhs_tile, j, rhs_tile, g):
                """One [128, 2048] output group: 4 matmuls + fused exp-rowsum."""
                nonlocal col
                ps_t = psp.tile([128, 2048], dt.float32, tag="ps")
                for q in range(4):
                    nc.tensor.matmul(
                        ps_t[:, q * 512:(q + 1) * 512],
                        lhs_tile[:, j * 128:(j + 1) * 128],
                        rhs_tile[:, g * 2048 + q * 512: g * 2048 + (q + 1) * 512],
                    )
                scr = scrp.tile([128, 2048], dt.float32, tag="scr")
                nc.scalar.activation(
                    scr[:],
                    ps_t[:],
                    mybir.ActivationFunctionType.Exp,
                    accum_out=sb_acc[:, col:col + 1],
                )
                col += 1

            # t2 cross part: 64 particle blocks x 2 x-halves
            for j in range(NP // 128):
                for g in range(XS // 2048):
                    group(sb_plhs, j, sb_xrhs, g)
            assert col == N_CROSS_GROUPS
            # t1 part: 20 (row-block, col-group) pairs; the row-block data
            # is packed consecutively in pslhs, so the lhsT index is just
            # the running slot while the rhs col-group follows T1_COUNTS.
            slot = 0
            for g in range(4):
                for _ in range(T1_COUNTS[g]):
                    group(sb_pslhs, slot, sb_prhs, g)
                    slot += 1
            assert col == N_GROUPS

            nc.sync.dma_start(acc_d[:], sb_acc[:])

    nc.compile()
    return nc


def _split_hi_lo(v):
    vh = v.astype(bf16)
    vl = (v - vh.astype(np.float32)).astype(bf16)
    return vh, vl


def _enc_lhsT(p):
    """p: [n, 16] f32 -> [K, n] bf16 stationary-side encoding."""
    n = p.shape[0]
    ph, pl = _split_hi_lo(np.ascontiguousarray(p, np.float32))
    p2 = (-GAMMA * (p.astype(np.float64) ** 2).sum(-1)).astype(np.float32)
    p2h, p2l = _split_hi_lo(p2)
    out = np.empty((K, n), bf16)
    out[0:16] = ph.T
    out[16:32] = pl.T
    out[32:48] = ph.T
    out[48:64] = pl.T
    out[64] = p2h
    out[65] = p2l
    out[66] = bf16(-GAMMA)
    out[67] = bf16(-GAMMA)
    return out


def _enc_rhs(u):
    """u: [n, 16] f32 -> [K, n] bf16 moving-side encoding."""
    n = u.shape[0]
    uh, ul = _split_hi_lo(np.ascontiguousarray(u, np.float32))
    u2 = ((u.astype(np.float64) ** 2).sum(-1)).astype(np.float32)
    u2h, u2l = _split_hi_lo(u2)
    out = np.empty((K, n), bf16)
    out[0:16] = uh.T
    out[16:32] = uh.T
    out[32:48] = ul.T
    out[48:64] = ul.T
    out[64] = bf16(1.0)
    out[65] = bf16(1.0)
    out[66] = u2h
    out[67] = u2l
    return out


_lock = threading.Lock()
_cached_nc = None


def _get_nc():
    global _cached_nc
    with _lock:
        if _cached_nc is None:
            _cached_nc = _build_nc()
        return _cached_nc


def _make_in_maps(x, particles):
    plhs = _enc_lhsT(particles)
    prhs = _enc_rhs(particles)
    in_maps = []
    for c in range(N_CORES):
        pairs = _t1_pairs(c)
        pslhs = np.concatenate(
            [plhs[:, r * 128:(r + 1) * 128] for r, _, _ in pairs], axis=1
        )
        in_maps.append(
            {
                "plhs": plhs,
                "prhs": prhs,
                "xrhs": _enc_rhs(x[c * XS:(c + 1) * XS]),
                "pslhs": np.ascontiguousarray(pslhs),
            }
        )
    return in_maps


def _combine(results):
    t2_sum = 0.0
    t1_sum = 0.0
    for c, r in enumerate(results):
        acc = r["acc"].astype(np.float64)
        t2_sum += acc[:, :N_CROSS_GROUPS].sum()
        w = np.array([w for _, _, w in _t1_pairs(c)], np.float64)
        t1_sum += (acc[:, N_CROSS_GROUPS:].sum(axis=0) * w).sum()
    t1 = t1_sum / (float(NP) * NP)
    t2 = 2.0 * t2_sum / (float(NX) * NP)
    return np.float32(t1 - t2)


def kernel(x, particles):
    x = np.asarray(x, np.float32)
    particles = np.asarray(particles, np.float32)
    assert x.shape == (NX, D) and particles.shape == (NP, D)

    nc = _get_nc()
    in_maps = _make_in_maps(x, particles)
    res = bass_utils.run_bass_kernel_spmd(nc, in_maps, core_ids=list(range(N_CORES)))
    return _combine(res.results)


# revision 4
# speedup vs baseline: 319.8693x; 319.8693x over previous
"""Trainium2 Bass kernel for the ExpCloudMMD loss.

reference math (gamma = 0.5):
  t1 = mean_{j,k} exp(-g*||p_j - p_k||^2)            over [8192, 8192]
  t2 = 2/(Nx*Np) * sum_{i,j} exp(-g*||x_i - p_j||^2) over [32768, 8192]
  out = t1 - t2  (f32 scalar)

Strategy (8 cores, SPMD, no collectives):
  - t2: shard x rows 8-way; each core computes its 4096x8192 cross block.
  - t1: the particle Gram is symmetric; in 2048x2048 super-blocks only the
    diagonal (4) + strict upper (6) of the 4x4 grid are computed, and the
    host doubles the upper sums. The 160 (row-block, col-group) pairs are
    dealt round-robin to the 8 cores; each core's pair list rides in its
    `pslhs` input tensor, so the program stays identical across cores.
  - The exp *argument* p.x - g|x|^2 - g|p|^2 is produced directly by a
    single K=68 matmul per output tile using an augmented bf16 hi/lo
    encoding (4-way split product + norm channels), so ScalarE needs no
    bias and the pipeline per [128, 2048] PSUM group is:
        4x matmul (PE) -> 1x activation(Exp, accum_out) (ACT)
    ACT is the roofline engine (~1 elem/lane/cycle @ 1.2 GHz).
  - Each ACT writes its partial row-sums into one column of a [128, 148]
    SBUF accumulator; the accumulator is DMA'd out and the final (tiny)
    weighted reduction + scaling happens on the host in float64.
"""

import threading

import ml_dtypes
import numpy as np

import concourse.bass as bass  # noqa: F401
import concourse.mybir as mybir
import concourse.tile as tile
from concourse import bacc, bass_utils

bf16 = ml_dtypes.bfloat16

GAMMA = 0.5
NX, NP, D = 32768, 8192, 16
N_CORES = 8
XS = NX // N_CORES     # 4096 x rows per core
K = 68                 # 4*16 (hi/lo product blocks) + 2 + 2 norm channels

# t1 coarse-triangle schedule: for col-super-group g (2048 particles),
# the computed row-blocks are the 16*(g+1) blocks of super-rows 0..g,
# dealt round-robin (r % 8) to cores -> per-core counts 2,4,6,8.
T1_COUNTS = [2, 4, 6, 8]
N_T1_GROUPS = sum(T1_COUNTS)                   # 20 per core
N_CROSS_GROUPS = (NP // 128) * (XS // 2048)    # 64 * 2 = 128 per core
N_GROUPS = N_CROSS_GROUPS + N_T1_GROUPS        # 148
PS_COLS = N_T1_GROUPS * 128                    # 2560 pslhs columns per core

N_PCHUNK = 8  # plhs load chunks (8 j-blocks each) for early compute start


def _t1_pairs(core):
    """[(row_block, col_group, weight)] for this core, in program order."""
    pairs = []
    for g in range(4):
        rows = [r for r in range(16 * (g + 1)) if r % N_CORES == core]
        assert len(rows) == T1_COUNTS[g]
        for r in rows:
            pairs.append((r, g, 1.0 if r // 16 == g else 2.0))
    return pairs


def _build_nc(repeats=1):
    nc = bacc.Bacc(
        "TRN2",
        target_bir_lowering=False,
        debug=False,
        enable_asserts=False,
        num_devices=N_CORES,
    )
    dt = mybir.dt
    plhs = nc.dram_tensor("plhs", [K, NP], dt.bfloat16, kind="ExternalInput").ap()
    prhs = nc.dram_tensor("prhs", [K, NP], dt.bfloat16, kind="ExternalInput").ap()
    xrhs = nc.dram_tensor("xrhs", [K, XS], dt.bfloat16, kind="ExternalInput").ap()
    pslhs = nc.dram_tensor("pslhs", [K, PS_COLS], dt.bfloat16, kind="ExternalInput").ap()
    acc_d = nc.dram_tensor("acc", [128, N_GROUPS], dt.float32, kind="ExternalOutput").ap()

    with tile.TileContext(nc) as tc:
        with (
            tc.tile_pool(name="const", bufs=1) as const,
            tc.tile_pool(name="scrp", bufs=2) as scrp,
            tc.tile_pool(name="psp", bufs=2, space="PSUM") as psp,
        ):
            sb_plhs = const.tile([K, NP], dt.bfloat16)
            sb_prhs = const.tile([K, NP], dt.bfloat16)
            sb_xrhs = const.tile([K, XS], dt.bfloat16)
            sb_pslhs = const.tile([K, PS_COLS], dt.bfloat16)
            sb_acc = const.tile([128, N_GROUPS], dt.float32)
            sb_tiny = const.tile([1, 1], dt.float32)

            # Warm the ACT exp table set (~2.7us) during the DMA prologue.
            nc.gpsimd.memset(sb_tiny[:], 0.0)
            nc.scalar.activation(
                sb_tiny[:], sb_tiny[:], mybir.ActivationFunctionType.Exp
            )

            # Input loads, in consumption order. The first matmul only
            # needs plhs chunk 0 + the first xrhs half.
            pchunk = NP // N_PCHUNK
            nc.sync.dma_start(sb_plhs[:, 0:pchunk], plhs[:, 0:pchunk])
            nc.sync.dma_start(sb_xrhs[:, 0:2048], xrhs[:, 0:2048])
            nc.sync.dma_start(sb_xrhs[:, 2048:XS], xrhs[:, 2048:XS])
            for i in range(1, N_PCHUNK):
                s = slice(i * pchunk, (i + 1) * pchunk)
                nc.sync.dma_start(sb_plhs[:, s], plhs[:, s])
            nc.sync.dma_start(sb_pslhs[:], pslhs[:])
            nc.sync.dma_start(sb_prhs[:], prhs[:])

            col = 0

            def group(lhs_tile, j, rhs_tile, g):
                """One [128, 2048] output group: 4 matmuls + fused exp-rowsum."""
                nonlocal col
                ps_t = psp.tile([128, 2048], dt.float32, tag="ps")
                for q in range(4):
                    nc.tensor.matmul(
                        ps_t[:, q * 512:(q + 1) * 512],
                        lhs_tile[:, j * 128:(j + 1) * 128],
                        rhs_tile[:, g * 2048 + q * 512: g * 2048 + (q + 1) * 512],
                    )
                scr = scrp.tile([128, 2048], dt.float32, tag="scr")
                nc.scalar.activation(
                    scr[:],
                    ps_t[:],
                    mybir.ActivationFunctionType.Exp,
                    accum_out=sb_acc[:, col:col + 1],
                )
                col += 1

            for _ in range(repeats):  # repeats>1 is a timing-only variant
                col = 0
                # t2 cross part: 64 particle blocks x 2 x-halves
                for j in range(NP // 128):
                    for g in range(XS // 2048):
                        group(sb_plhs, j, sb_xrhs, g)
                assert col == N_CROSS_GROUPS
                # t1 part: 20 (row-block, col-group) pairs; the row-block
                # data is packed consecutively in pslhs, so the lhsT index
                # is the running slot while the rhs col-group follows
                # T1_COUNTS.
                slot = 0
                for g in range(4):
                    for _ in range(T1_COUNTS[g]):
                        group(sb_pslhs, slot, sb_prhs, g)
                        slot += 1
                assert col == N_GROUPS

            nc.sync.dma_start(acc_d[:], sb_acc[:])

    nc.compile()
    return nc


def _split_hi_lo(v):
    vh = v.astype(bf16)
    vl = (v - vh.astype(np.float32)).astype(bf16)
    return vh, vl


def _enc_lhsT(p):
    """p: [n, 16] f32 -> [K, n] bf16 stationary-side encoding."""
    n = p.shape[0]
    ph, pl = _split_hi_lo(np.ascontiguousarray(p, np.float32))
    p2 = (-GAMMA * (p.astype(np.float64) ** 2).sum(-1)).astype(np.float32)
    p2h, p2l = _split_hi_lo(p2)
    out = np.empty((K, n), bf16)
    out[0:16] = ph.T
    out[16:32] = pl.T
    out[32:48] = ph.T
    out[48:64] = pl.T
    out[64] = p2h
    out[65] = p2l
    out[66] = bf16(-GAMMA)
    out[67] = bf16(-GAMMA)
    return out


def _enc_rhs(u):
    """u: [n, 16] f32 -> [K, n] bf16 moving-side encoding."""
    n = u.shape[0]
    uh, ul = _split_hi_lo(np.ascontiguousarray(u, np.float32))
    u2 = ((u.astype(np.float64) ** 2).sum(-1)).astype(np.float32)
    u2h, u2l = _split_hi_lo(u2)
    out = np.empty((K, n), bf16)
    out[0:16] = uh.T
    out[16:32] = uh.T
    out[32:48] = ul.T
    out[48:64] = ul.T
    out[64] = bf16(1.0)
    out[65] = bf16(1.0)
    out[66] = u2h
    out[67] = u2l
    return out


_lock = threading.Lock()
_cached_nc = None


def _get_nc():
    global _cached_nc
    with _lock:
        if _cached_nc is None:
            _cached_nc = _build_nc()
        return _cached_nc


def _make_in_maps(x, particles):
    plhs = _enc_lhsT(particles)
    prhs = _enc_rhs(particles)
    in_maps = []
    for c in range(N_CORES):
        pairs = _t1_pairs(c)
        pslhs = np.concatenate(
            [plhs[:, r * 128:(r + 1) * 128] for r, _, _ in pairs], axis=1
        )
        in_maps.append(
            {
                "plhs": plhs,
                "prhs": prhs,
                "xrhs": _enc_rhs(x[c * XS:(c + 1) * XS]),
                "pslhs": np.ascontiguousarray(pslhs),
            }
        )
    return in_maps


def _combine(results):
    t2_sum = 0.0
    t1_sum = 0.0
    for c, r in enumerate(results):
        acc = r["acc"].astype(np.float64)
        t2_sum += acc[:, :N_CROSS_GROUPS].sum()
        w = np.array([w for _, _, w in _t1_pairs(c)], np.float64)
        t1_sum += (acc[:, N_CROSS_GROUPS:].sum(axis=0) * w).sum()
    t1 = t1_sum / (float(NP) * NP)
    t2 = 2.0 * t2_sum / (float(NX) * NP)
    return np.float32(t1 - t2)


def kernel(x, particles):
    x = np.asarray(x, np.float32)
    particles = np.asarray(particles, np.float32)
    assert x.shape == (NX, D) and particles.shape == (NP, D)

    nc = _get_nc()
    in_maps = _make_in_maps(x, particles)
    res = bass_utils.run_bass_kernel_spmd(nc, in_maps, core_ids=list(range(N_CORES)))
    return _combine(res.results)
